# revision 22
# baseline (speedup 1.0000x reference)
"""CTBG circuit kernel for Trainium2, data-parallel over batch on 8 NeuronCores.

Network (per reference):
  gpe_out = x @ (gpe_w * gpe_mask.T) + gpe_b              [B, 1536]
  gpi_in  = concat([x, gpe_out], -1)                      [B, 3072]
  gpi_out = gpi_in @ (gpi_w * gpi_mask.T) + gpi_b         [B, 1536]
  h1 = relu(gpi_out @ w1 + b1); h2 = relu(h1 @ w2 + b2)
  out = relu(h2 @ w3 + b3)                                [B, 6]

Key algebraic identity: gpe_out and gpi_out feed forward with no
intervening nonlinearity, so the masked front end folds into one
[1536, 512] weight computed ON DEVICE once per launch:

  mw_gpe = gpe_w * gpe_mask.T
  mw_gpi = gpi_w * gpi_mask.T
  M      = mw_gpi[1536:] @ w1                       [1536, 512]
  Wfold  = mw_gpi[:1536] @ w1 + mw_gpe @ M          [1536, 512]
  bfold  = gpe_b @ M + gpi_b @ w1 + b1              [512]
  h1 = relu(x @ Wfold + bfold)   -> h2 -> out       (per batch row)

The fold is SHARDED across the 8 cores: core c computes rows
[c*192, (c+1)*192) of M (then of Wfold) from host-sliced mask/weight
COLUMN slices, assembled with two bf16 DRAM AllGathers. F2's x-part
(independent of M) runs during the M gather; the bias fold runs during
the Wfold gather.

Queue discipline: the x activations stream on the gpsimd (SWDGE) queue
only, so the latency-critical fold loads, slice writes, and gather
readbacks on the sync (HWDGE) queue never wait behind them.

All fold operands arrive as host-PACKED tile-major [128, wide] bf16
tensors so every DMA moves multi-KB contiguous rows. Host prep is
layout/dtype only (bf16 casts, transposes, slicing, packing) — zero
network FLOPs.
"""

import numpy as np
import ml_dtypes

BF = ml_dtypes.bfloat16

NCORES = 8
B = 16384
BS = B // NCORES          # 2048 rows per core
BT = 512                  # batch tile (matmul free dim)
NBT = BS // BT            # 4
D1 = 1536                 # gpe input dim (x features)
H = 512                   # mlp hidden
A = 6                     # action dim
SL = D1 // NCORES         # 192: fold rows per core

NI = D1 // 128            # 12 i-chunks
NU = D1 // 128            # 12 u-chunks
NV = D1 // 128            # 12 v-chunks
NH = H // 128             # 4 h-chunks

GW = 2 * SL               # 384: gpi slice width per v (islice|uslice)
EW = SL                   # 192: gpe slice width per u

_CACHE = {}


def _build():
    import concourse.bacc as bacc
    import concourse.tile as tile
    from concourse import mybir
    from concourse.masks import make_identity

    FP32 = mybir.dt.float32
    BF16 = mybir.dt.bfloat16
    Act = mybir.ActivationFunctionType

    nc = bacc.Bacc(None, num_devices=NCORES)

    w1pk_d = nc.dram_tensor("w1pk", [128, NV * H], BF16, kind="ExternalInput")
    gpipk_d = nc.dram_tensor("gpipk", [128, NV * 2 * GW], BF16,
                             kind="ExternalInput")
    gpepk_d = nc.dram_tensor("gpepk", [128, NU * 2 * EW], BF16,
                             kind="ExternalInput")
    mlppk_d = nc.dram_tensor("mlppk", [128, NH * (H + A)], BF16,
                             kind="ExternalInput")
    xpk_d = nc.dram_tensor("xpk", [128, NI * BS], BF16, kind="ExternalInput")
    gpeb_d = nc.dram_tensor("gpe_b", [D1], FP32, kind="ExternalInput")
    gpib_d = nc.dram_tensor("gpi_b", [D1], FP32, kind="ExternalInput")
    b1_d = nc.dram_tensor("b1", [H], FP32, kind="ExternalInput")
    b2_d = nc.dram_tensor("b2", [H], FP32, kind="ExternalInput")
    b3_d = nc.dram_tensor("b3", [A], FP32, kind="ExternalInput")
    o_d = nc.dram_tensor("out", [A, BS], FP32, kind="ExternalOutput")

    RG = [list(range(NCORES))]
    GROUPS = [(0, 128), (128, SL - 128)]

    with tile.TileContext(nc) as tc:
        with (
            tc.tile_pool(name="wp", bufs=1) as wp,
            tc.tile_pool(name="ap", bufs=1) as ap,
            tc.tile_pool(name="a2", bufs=2) as a2,
            tc.tile_pool(name="dp", bufs=1, space="DRAM") as dp,
            tc.tile_pool(name="psp", bufs=3, space="PSUM") as psp,
            tc.tile_pool(name="psf", bufs=1, space="PSUM") as psfp,
            tc.tile_pool(name="ps2", bufs=1, space="PSUM") as ps2p,
            tc.tile_pool(name="pso", bufs=1, space="PSUM") as psop,
            tc.tile_pool(name="pst", bufs=1, space="PSUM") as pstp,
        ):
            # ---- critical loads on sync; x on gpsimd only
            w1pk = wp.tile([128, NV * H], BF16, tag="w1pk")
            nc.sync.dma_start(out=w1pk[:, :], in_=w1pk_d[:, :])
            w1t = [w1pk[:, v * H:(v + 1) * H] for v in range(NV)]

            gpipk = wp.tile([128, NV * 2 * GW], BF16, tag="gpipk")
            hwid = NV * GW
            nc.sync.dma_start(out=gpipk[:, 0:hwid], in_=gpipk_d[:, 0:hwid])
            nc.sync.dma_start(out=gpipk[:, hwid:2 * hwid],
                              in_=gpipk_d[:, hwid:2 * hwid])
            mwgpi = []
            for v in range(NV):
                mk = gpipk[:, v * 2 * GW:v * 2 * GW + GW]
                wt = gpipk[:, v * 2 * GW + GW:(v + 1) * 2 * GW]
                t = wp.tile([128, GW], BF16, tag=f"mwgpi{v}", name=f"mwgpi{v}")
                nc.vector.tensor_mul(t[:, :], mk, wt)
                mwgpi.append(t)

            gpepk = wp.tile([128, NU * 2 * EW], BF16, tag="gpepk")
            nc.sync.dma_start(out=gpepk[:, :], in_=gpepk_d[:, :])
            mwgpe = []
            for u in range(NU):
                mk = gpepk[:, u * 2 * EW:u * 2 * EW + EW]
                wt = gpepk[:, u * 2 * EW + EW:(u + 1) * 2 * EW]
                t = wp.tile([128, EW], BF16, tag=f"mwgpe{u}", name=f"mwgpe{u}")
                nc.vector.tensor_mul(t[:, :], mk, wt)
                mwgpe.append(t)

            mlppk = wp.tile([128, NH * (H + A)], BF16, tag="mlppk")
            nc.sync.dma_start(out=mlppk[:, :], in_=mlppk_d[:, :])
            w2t = [mlppk[:, k * H:(k + 1) * H] for k in range(NH)]
            w3t = [mlppk[:, NH * H + k * A:NH * H + (k + 1) * A]
                   for k in range(NH)]

            xpk = wp.tile([128, NI * BS], BF16, tag="xpk")
            xq = NI * BS // 4    # one t-major quarter = one batch tile

            def xload(q_i):
                nc.gpsimd.dma_start(out=xpk[:, q_i * xq:(q_i + 1) * xq],
                                    in_=xpk_d[:, q_i * xq:(q_i + 1) * xq])

            def xsl(i, t_i):
                return xpk[:, t_i * NI * BT + i * BT:t_i * NI * BT + (i + 1) * BT]

            xload(0)

            ident = wp.tile([128, 128], FP32, tag="ident")
            make_identity(nc, ident[:, :])

            def load_bias_cols(b_dram, n, tag):
                nat = wp.tile([n, 128], FP32, tag=f"{tag}_nat")
                nc.sync.dma_start(out=nat[:, :],
                                  in_=b_dram.rearrange("(c p) -> c p", p=128))
                ps = pstp.tile([128, NU], FP32, tag="pst")
                nc.tensor.transpose(ps[:, 0:n], nat[:, :], ident[0:n, 0:n])
                sb = wp.tile([128, n], FP32, tag=tag)
                nc.vector.tensor_copy(sb[:, :], ps[:, 0:n])
                return sb

            gpeb_sb = load_bias_cols(gpeb_d, NU, "gpeb")
            gpib_sb = load_bias_cols(gpib_d, NV, "gpib")
            b2_sb = load_bias_cols(b2_d, NH, "b2sb")
            gpeb_bf = wp.tile([128, NU], BF16, tag="gpebf")
            nc.vector.tensor_copy(gpeb_bf[:, :], gpeb_sb[:, :])
            gpib_bf = wp.tile([128, NV], BF16, tag="gpibf")
            nc.vector.tensor_copy(gpib_bf[:, :], gpib_sb[:, :])
            b1row = wp.tile([1, H], FP32, tag="b1row")
            nc.sync.dma_start(out=b1row[:, :],
                              in_=b1_d.rearrange("(one h) -> one h", one=1))
            b3_sb = wp.tile([A, 1], FP32, tag="b3sb")
            nc.sync.dma_start(out=b3_sb[:, :],
                              in_=b3_d.rearrange("(a one) -> a one", one=1))

            # ---- F1s: this core's 192 M rows
            msl_dram = dp.tile([SL, H], BF16, tag="msl")
            for g, (r0, rn) in enumerate(GROUPS):
                ps = psp.tile([128, H], FP32, tag="ps")
                for v in range(NV):
                    nc.tensor.matmul(ps[0:rn, :],
                                     mwgpi[v][:, SL + r0:SL + r0 + rn],
                                     w1t[v][:, :],
                                     start=(v == 0), stop=(v == NV - 1))
                sb = ap.tile([128, H], BF16, tag=f"mslice{g}")
                nc.scalar.activation(sb[0:rn, :], ps[0:rn, :], Act.Copy)
                nc.sync.dma_start(out=msl_dram[r0:r0 + rn, :], in_=sb[0:rn, :])

            mfull = dp.tile([D1, H], BF16, tag="mfull")
            nc.gpsimd.collective_compute(
                "AllGather", mybir.AluOpType.bypass, replica_groups=RG,
                ins=[msl_dram[:, :].opt()], outs=[mfull[:, :].opt()])
            xload(1)

            # ---- F2s x-part first: independent of M, hides the M gather
            psf = []
            for g, (r0, rn) in enumerate(GROUPS):
                ps = psfp.tile([128, H], FP32, tag=f"psf{g}")
                psf.append(ps)
                for v in range(NV):
                    nc.tensor.matmul(ps[0:rn, :],
                                     mwgpi[v][:, r0:r0 + rn],
                                     w1t[v][:, :],
                                     start=(v == 0), stop=False)

            # M readback on the uncontended sync queue
            Mt = []
            for u in range(NU):
                t = wp.tile([128, H], BF16, tag=f"M{u}")
                nc.sync.dma_start(out=t[:, :],
                                  in_=mfull[u * 128:(u + 1) * 128, :])
                Mt.append(t)

            wfs_dram = dp.tile([SL, H], BF16, tag="wfs")
            for g, (r0, rn) in enumerate(GROUPS):
                ps = psf[g]
                for u in range(NU):
                    nc.tensor.matmul(ps[0:rn, :],
                                     mwgpe[u][:, r0:r0 + rn],
                                     Mt[u][:, :],
                                     start=False, stop=(u == NU - 1))
                sb = ap.tile([128, H], BF16, tag=f"wfslice{g}")
                nc.scalar.activation(sb[0:rn, :], ps[0:rn, :], Act.Copy)
                nc.sync.dma_start(out=wfs_dram[r0:r0 + rn, :], in_=sb[0:rn, :])

            wff = dp.tile([D1, H], BF16, tag="wff")
            nc.gpsimd.collective_compute(
                "AllGather", mybir.AluOpType.bypass, replica_groups=RG,
                ins=[wfs_dram[:, :].opt()], outs=[wff[:, :].opt()])
            xload(2)
            xload(3)

            # ---- bias fold on the PE while the Wf gather runs
            psb = ps2p.tile([1, H], FP32, tag="psb")
            for v in range(NV):
                nc.tensor.matmul(psb[:, :], gpib_bf[:, v:v + 1], w1t[v][:, :],
                                 start=(v == 0), stop=False)
            for u in range(NU):
                nc.tensor.matmul(psb[:, :], gpeb_bf[:, u:u + 1], Mt[u][:, :],
                                 start=False, stop=(u == NU - 1))
            brow = wp.tile([1, H], FP32, tag="brow")
            nc.vector.tensor_add(brow[:, :], psb[:, :], b1row[:, :])
            bfold = wp.tile([128, NH], FP32, tag="bfold")
            for c in range(NH):
                ps = pstp.tile([128, NV], FP32, tag="pst")
                nc.tensor.transpose(ps[:, 0:1],
                                    brow[0:1, c * 128:(c + 1) * 128],
                                    ident[0:1, 0:1])
                nc.scalar.activation(bfold[:, c:c + 1], ps[:, 0:1], Act.Copy)

            Wf = []
            for i in range(NI):
                t = wp.tile([128, H], BF16, tag=f"Wf{i}")
                nc.sync.dma_start(out=t[:, :],
                                  in_=wff[i * 128:(i + 1) * 128, :])
                Wf.append(t)

            # ---- B: batch pass, t-outer (tile t needs only x quarter t)
            for t_i in range(NBT):
                h1 = []
                for hc in range(NH):
                    ps = psp.tile([128, BT], FP32, tag="ps")
                    for i in range(NI):
                        nc.tensor.matmul(ps[:, :],
                                         Wf[i][:, hc * 128:(hc + 1) * 128],
                                         xsl(i, t_i),
                                         start=(i == 0), stop=(i == NI - 1))
                    ht = ap.tile([128, BT], BF16, tag=f"h1_{hc}", bufs=2)
                    nc.scalar.activation(ht[:, :], ps[:, :], Act.Relu,
                                         bias=bfold[:, hc:hc + 1])
                    h1.append(ht)

                h2 = []
                for mc in range(NH):
                    ps = psp.tile([128, BT], FP32, tag="ps")
                    for k in range(NH):
                        nc.tensor.matmul(ps[:, :],
                                         w2t[k][:, mc * 128:(mc + 1) * 128],
                                         h1[k][:, :],
                                         start=(k == 0), stop=(k == NH - 1))
                    ht = a2.tile([128, BT], BF16, tag=f"h2_{mc}")
                    nc.scalar.activation(ht[:, :], ps[:, :], Act.Relu,
                                         bias=b2_sb[:, mc:mc + 1])
                    h2.append(ht)

                pso = psop.tile([A, BT], FP32, tag="pso")
                for k in range(NH):
                    nc.tensor.matmul(pso[:, :], w3t[k][:, :], h2[k][:, :],
                                     start=(k == 0), stop=(k == NH - 1))
                osb = a2.tile([A, BT], FP32, tag="osb")
                nc.scalar.activation(osb[:, :], pso[:, :], Act.Relu,
                                     bias=b3_sb[:, 0:1])
                nc.sync.dma_start(out=o_d[:, t_i * BT:(t_i + 1) * BT],
                                  in_=osb[:, :])

    nc.finalize()
    return nc


def _get_nc():
    if "nc" not in _CACHE:
        _CACHE["nc"] = _build()
    return _CACHE["nc"]


def _tilepack(mat, p=128):
    """[n*p, W] row-major -> [p, n*W] tile-major."""
    n = mat.shape[0] // p
    return np.ascontiguousarray(
        mat.reshape(n, p, -1).transpose(1, 0, 2).reshape(p, -1))


def _xpack(xs):
    """[1536, 2048] -> [128, 24576] t-major: col = t*6144 + i*512 + c."""
    return np.ascontiguousarray(
        xs.reshape(NI, 128, NBT, BT).transpose(1, 2, 0, 3).reshape(128, -1))


def _prep_inputs(inputs):
    """Host-side layout/dtype prep only (no network FLOPs)."""
    f = {k: np.asarray(v) for k, v in inputs.items()}
    xT = np.ascontiguousarray(f["x"].astype(BF).T)            # [1536, B]
    gpem = f["gpe_mask"].astype(BF)                           # [u, i]
    gpewT = np.ascontiguousarray(f["gpe_w"].astype(BF).T)     # [u, i]
    gpim = f["gpi_mask"].astype(BF)                           # [v, j]
    gpiwT = np.ascontiguousarray(f["gpi_w"].astype(BF).T)     # [v, j]

    shared = {
        "w1pk": _tilepack(f["w1"].astype(BF)),
        "mlppk": np.ascontiguousarray(np.concatenate(
            [_tilepack(f["w2"].astype(BF)), _tilepack(f["w3"].astype(BF))],
            axis=1)),
        "gpe_b": np.ascontiguousarray(f["gpe_b"], dtype=np.float32),
        "gpi_b": np.ascontiguousarray(f["gpi_b"], dtype=np.float32),
        "b1": np.ascontiguousarray(f["b1"], dtype=np.float32),
        "b2": np.ascontiguousarray(f["b2"], dtype=np.float32),
        "b3": np.ascontiguousarray(f["b3"], dtype=np.float32),
    }

    def interleave_pack(mask, wT, cols):
        a = mask[:, cols].reshape(NV, 128, -1)
        b = wT[:, cols].reshape(NV, 128, -1)
        return np.ascontiguousarray(
            np.concatenate([a, b], axis=2).transpose(1, 0, 2).reshape(128, -1))

    in_maps = []
    for c in range(NCORES):
        icols = np.arange(c * SL, (c + 1) * SL)
        ucols = np.arange(D1 + c * SL, D1 + (c + 1) * SL)
        gcols = np.concatenate([icols, ucols])
        in_maps.append(dict(
            shared,
            xpk=_xpack(xT[:, c * BS:(c + 1) * BS]),
            gpipk=interleave_pack(gpim, gpiwT, gcols),
            gpepk=interleave_pack(gpem, gpewT, icols),
        ))
    return in_maps


def _run(inputs, trace=False):
    from concourse.bass_utils import run_bass_kernel_spmd

    nc = _get_nc()
    in_maps = _prep_inputs(inputs)
    res = run_bass_kernel_spmd(nc, in_maps, list(range(NCORES)), trace=trace)
    out = np.concatenate(
        [np.asarray(res.results[c]["out"]).T for c in range(NCORES)], axis=0)
    return out.astype(np.float32), res


def kernel(**inputs):
    out, _ = _run(inputs, trace=False)
    return out
